# revision 8
# baseline (speedup 1.0000x reference)
"""GNN message-passing (2-layer GraphConv + fetch/linear) on 8 TRN2 NeuronCores.

Strategy (self-contained; shapes hardcoded for the target problem):
  - Nodes dst-sharded across 8 cores (25000/core). Each core owns the edges
    whose dst falls in its shard; the full feature table is replicated in
    DRAM so the per-edge gather h[src] is core-local.
  - The dst-side norm is folded into the tables on the host
    (table0 = features*norm; table1 = h1*norm by construction), so the
    per-chunk indicator is a plain is_equal (no per-edge norm multiply).
  - Edges are sorted per core by (group g of 8 superblocks, src-owner o,
    superblock sb of 256 dst slots). Only the (g, o) runs are padded (to the
    max across cores, 128-aligned) so all 8 cores run one SPMD program;
    128-edge chunks may span superblock boundaries inside a run. For each
    chunk x superblock incidence (the union across cores), one DVE is_equal
    builds a [128, 256] indicator from a group-local slot id, and one PE
    matmul accumulates aggT[f, slot] += X_chunk.T @ ind into that
    superblock's PSUM tile. Out-of-window slots simply never match, so a
    core lacking edges there contributes zero.
  - dma_gather (int16 indices, one 256B row per edge, up to 4096 rows per
    call) pulls src rows into SBUF. Index/slot slabs are SBUF-resident and
    shared by both layers.
  - Per 128-dst block: R = aggT_b.T @ W in PSUM, then one ScalarE
    activation relu(R * scale) writes the next-layer table row block
    (scale folds the dst-side norm; layer 1 stores h1*norm via scale=norm^2).
  - One AllGather replicates h1*norm to all cores between the layers.
    Final stage: dma_gather of the fetched rows, transpose, matmul with
    w3^T, bias.
"""

import os
import sys

sys.path.insert(0, "/opt/trn_rl_repo")

import numpy as np

import concourse.bacc as bacc
import concourse.bass as bass
import concourse.mybir as mybir
import concourse.tile as tile
from concourse.bass_utils import run_bass_kernel_spmd
from concourse.library_config import mlp as mlp_lib

# ---------------------------------------------------------------- config

N_NODES = 200000
N_EDGES = 3200000
NUM_GRAPHS = 100
NODES_PER_GRAPH = 2000
D = 128
D_OUT = 64
NC = 8                       # cores
S = N_NODES // NC            # 25000 dst nodes per core
SB = 512                     # dst slots per superblock (one PSUM bank fp32)
NSB = (S + SB - 1) // SB     # 49
S_PAD = NSB * SB             # 25088
G_SB = 4                     # superblocks per gather group
NG = (NSB + G_SB - 1) // G_SB  # 13
GSLOT = G_SB * SB            # 2048 dst slots per group
NBLK = S_PAD // 128          # 196 dst blocks per core
NRUN = NG * NC               # (g, o) runs
MAX_CALL = int(os.environ.get("MAX_CALL", "2048"))  # rows per dma_gather call
N_QUEUES = 4

TBL_DT = mybir.dt.float16    # table / gather / indicator dtype
TBL_NP = np.float16
ACC_DT = mybir.dt.float32

last_result = None           # BassKernelResults of the most recent run


def _roundup(x, m):
    return (x + m - 1) // m * m


# ---------------------------------------------------------------- host plan


class Plan:
    """Host-side reorganization of the edge list into the SPMD structure."""

    def __init__(self, src, dst):
        src = np.asarray(src).astype(np.int64)
        dst = np.asarray(dst).astype(np.int64)
        deg = np.bincount(dst, minlength=N_NODES).astype(np.float64)
        self.norm = (1.0 / np.sqrt(np.clip(deg, 1.0, None))).astype(np.float32)

        owner = src // S
        core = dst // S
        dloc = dst - core * S
        sbg = dloc // SB                 # superblock 0..NSB-1
        g = sbg // G_SB                  # group 0..NG-1
        sbl = sbg - g * G_SB             # superblock within group
        slot_g = dloc - g * GSLOT        # group-local slot 0..GSLOT-1

        runid = g * NC + owner
        cellid = runid * G_SB + sbl
        ncell = NRUN * G_SB

        cellcnt = np.zeros((NC, ncell), np.int64)
        for c in range(NC):
            cellcnt[c] = np.bincount(cellid[core == c], minlength=ncell)
        runcnt = cellcnt.reshape(NC, NRUN, G_SB).sum(axis=2)
        target_run = _roundup(runcnt.max(axis=0), 128)          # [NRUN]
        run_base = np.zeros(NRUN, np.int64)
        run_base[1:] = np.cumsum(target_run)[:-1]
        self.E_pad = int(target_run.sum())
        self.target_run = target_run
        self.run_base = run_base

        # per-core cell start offsets inside the (padded, shared) run
        cell_off = np.zeros((NC, NRUN, G_SB), np.int64)
        cc = cellcnt.reshape(NC, NRUN, G_SB)
        cell_off[:, :, 1:] = np.cumsum(cc, axis=2)[:, :, :-1]
        cell_start = run_base[None, :, None] + cell_off        # [NC,NRUN,G_SB]
        cell_end = cell_start + cc

        # per-core padded edge arrays
        self.idx16 = np.zeros((NC, self.E_pad), np.int16)
        self.slot = np.full((NC, self.E_pad), -1.0, np.float32)
        starts_flat = cell_start.reshape(NC, ncell)
        for c in range(NC):
            m = core == c
            cid = cellid[m]
            srt = np.argsort(cid, kind="stable")
            cid_s = cid[srt]
            pref = np.zeros(ncell + 1, np.int64)
            pref[1:] = np.cumsum(cellcnt[c])
            ranks = np.arange(cid_s.size) - pref[cid_s]
            pos = starts_flat[c, cid_s] + ranks
            es = src[m][srt]
            self.idx16[c, pos] = (es - (es // S) * S).astype(np.int16)
            self.slot[c, pos] = slot_g[m][srt].astype(np.float32)

        # union chunk x superblock incidence (core-invariant op list)
        # ops[g] = ordered list of (t, sbl, is_first, is_last)
        nchunk = self.E_pad // 128
        inc = {}                          # (t, sbl) -> True
        for c in range(NC):
            st = cell_start[c]
            en = cell_end[c]
            for r in range(NRUN):
                for s_ in range(G_SB):
                    if en[r, s_] <= st[r, s_]:
                        continue
                    t0 = st[r, s_] // 128
                    t1 = (en[r, s_] + 127) // 128
                    for t in range(t0, t1):
                        inc[(t, s_)] = True

        chunk_run = np.zeros(nchunk, np.int64)
        for r in range(NRUN):
            c0 = run_base[r] // 128
            c1 = (run_base[r] + target_run[r]) // 128
            chunk_run[c0:c1] = r

        self.ops = [[] for _ in range(NG)]
        first_seen = {}
        last_idx = {}
        for gg in range(NG):
            seq = []
            for o in range(NC):
                r = gg * NC + o
                c0 = run_base[r] // 128
                c1 = (run_base[r] + target_run[r]) // 128
                for t in range(c0, c1):
                    for s_ in range(G_SB):
                        if (t, s_) in inc:
                            seq.append((t, s_))
            for i, (t, s_) in enumerate(seq):
                key = s_
                if key not in first_seen:
                    first_seen[key] = i
                last_idx[key] = i
            ops = []
            for i, (t, s_) in enumerate(seq):
                ops.append((t, s_, i == first_seen[s_], i == last_idx[s_]))
            self.ops[gg] = ops
            first_seen.clear()
            last_idx.clear()

        # gather calls: per run split into <=MAX_CALL pieces
        self.calls = [[] for _ in range(NG)]   # (o, start, n)
        for gg in range(NG):
            for o in range(NC):
                r = gg * NC + o
                start = int(run_base[r])
                left = int(target_run[r])
                while left > 0:
                    n = min(MAX_CALL, left)
                    self.calls[gg].append((o, start, n))
                    start += n
                    left -= n

        # chunk -> (call order index within group, k) for lhsT slicing
        self.chunk_call = {}
        for gg in range(NG):
            for ci, (o, start, n) in enumerate(self.calls[gg]):
                for k in range(n // 128):
                    self.chunk_call[start // 128 + k] = (gg, ci, k)

    def wrap_idx(self, c):
        a = self.idx16[c].reshape(-1, 16).T  # [16, E/16]
        return np.tile(a, (8, 1)).copy()

    def chunk_cols(self, arr_c, dtype):
        return arr_c.reshape(-1, 128).T.astype(dtype).copy()  # [128, E/128]


# ---------------------------------------------------------------- bass build


def _emit_layer(nc, plan, pools, consts, table, win_rows, bounce, w_tile,
                scale_name, slabs, qoff=0):
    gp, indp, aggp, aggsbp, rp, stp = (pools[k] for k in
                                       ("g", "ind", "agg", "aggsb", "r", "st"))
    iota_t = consts["iota"]
    scale_t = consts[scale_name]
    idx_t, slot_t = slabs

    qn = qoff
    for gg in range(NG):
        # gather calls for this group
        gtiles = []
        for (o, start, n) in plan.calls[gg]:
            gt = gp.tile([128, n // 128, D], TBL_DT, tag="g")
            nc.gpsimd.dma_gather(
                gt[:, : n // 128, :],
                table[o * win_rows: o * win_rows + win_rows, :],
                idx_t[:, start // 16: (start + n) // 16],
                n, n, D, queue_num=qn % N_QUEUES)
            qn += 1
            gtiles.append((start // 128, gt))

        agg_tiles = {}  # keyed by superblock; one PSUM bank each
        for (t, s_, is_first, is_last) in plan.ops[gg]:
            _, ci, k = plan.chunk_call[t]
            gt = gtiles[ci][1]
            ind = indp.tile([128, SB], TBL_DT, tag="ind")
            nc.vector.tensor_scalar(
                ind[:], iota_t[:, s_ * SB:(s_ + 1) * SB],
                slot_t[:, t:t + 1], None, mybir.AluOpType.is_equal)
            if s_ not in agg_tiles:
                agg_tiles[s_] = aggp.tile([128, SB], ACC_DT, tag="agg",
                                          name=f"agg_g{gg}_s{s_}")
            nc.tensor.matmul(agg_tiles[s_][:], lhsT=gt[:, k, :], rhs=ind[:],
                             start=is_first, stop=is_last)
            if is_last:
                # drain this superblock
                aggT = aggsbp.tile([128, SB], TBL_DT, tag="aggsb")
                nc.scalar.activation(aggT[:], agg_tiles[s_][:],
                                     mybir.ActivationFunctionType.Copy)
                stage = stp.tile([128, SB // 128, D], TBL_DT, tag="st")
                for b in range(SB // 128):
                    blk = (gg * GSLOT + s_ * SB) // 128 + b
                    r = rp.tile([128, D], ACC_DT, tag="r")
                    nc.tensor.matmul(r[:], lhsT=aggT[:, b * 128:(b + 1) * 128],
                                     rhs=w_tile[:], start=True, stop=True)
                    nc.scalar.activation(stage[:, b, :], r[:],
                                         mybir.ActivationFunctionType.Relu,
                                         scale=scale_t[:, blk:blk + 1])
                base = gg * GSLOT + s_ * SB
                nc.sync.dma_start(
                    bounce[base:base + SB, :].rearrange(
                        "(c p) f -> p c f", p=128),
                    stage[:])
    return qn


def build_bass(plan, weights):
    nc = bacc.Bacc("TRN2", target_bir_lowering=False,
                   num_swdge_queues=N_QUEUES)
    w1, b1, w2, b2, w3, b3 = weights
    assert abs(b1).max() == 0 and abs(b2).max() == 0, \
        "nonzero conv bias not supported by this build"

    E_pad = plan.E_pad
    table0 = nc.dram_tensor("table0", [N_NODES, D], TBL_DT,
                            kind="ExternalInput")
    idx_d = nc.dram_tensor("idx", [128, E_pad // 16], mybir.dt.int16,
                           kind="ExternalInput")
    slot_d = nc.dram_tensor("slot", [128, E_pad // 128], mybir.dt.float32,
                            kind="ExternalInput")
    iota_d = nc.dram_tensor("iota", [128, GSLOT], TBL_DT, kind="ExternalInput")
    norm_d = nc.dram_tensor("normsc", [128, NBLK], mybir.dt.float32,
                            kind="ExternalInput")
    norm2_d = nc.dram_tensor("normsc2", [128, NBLK], mybir.dt.float32,
                             kind="ExternalInput")
    w1_d = nc.dram_tensor("w1t", [D, D], TBL_DT, kind="ExternalInput")
    w2_d = nc.dram_tensor("w2t", [D, D], TBL_DT, kind="ExternalInput")
    w3_d = nc.dram_tensor("w3t", [D, D_OUT], TBL_DT, kind="ExternalInput")
    b3_d = nc.dram_tensor("b3c", [D_OUT, 1], mybir.dt.float32,
                          kind="ExternalInput")
    ident_d = nc.dram_tensor("ident", [128, 128], TBL_DT,
                             kind="ExternalInput")
    fidx_d = nc.dram_tensor("fidx", [128, 8], mybir.dt.int16,
                            kind="ExternalInput")
    fcnt_d = nc.dram_tensor("fcnt", [1, 1], mybir.dt.uint32,
                            kind="ExternalInput")
    y_d = nc.dram_tensor("y", [D_OUT, 128], mybir.dt.float32,
                         kind="ExternalOutput")

    h1_bounce = nc.dram_tensor("h1b", [S_PAD, D], TBL_DT)
    table1 = nc.dram_tensor("table1", [NC * S_PAD, D], TBL_DT,
                            addr_space="Shared")
    h2_loc = nc.dram_tensor("h2loc", [S_PAD, D], TBL_DT)

    with tile.TileContext(nc) as tc:
        nc.gpsimd.load_library(mlp_lib)
        with (
            tc.tile_pool(name="consts", bufs=1) as cpool,
            tc.tile_pool(name="slab", bufs=1) as slabp,
            tc.tile_pool(name="g", bufs=3) as gp,
            tc.tile_pool(name="ind", bufs=8) as indp,
            tc.tile_pool(name="aggsb", bufs=4) as aggsbp,
            tc.tile_pool(name="st", bufs=4) as stp,
            tc.tile_pool(name="agg", bufs=6, space="PSUM") as aggp,
            tc.tile_pool(name="r", bufs=2, space="PSUM") as rp,
        ):
            consts = {}
            for nm, dr, shape, dt in (
                ("iota", iota_d, [128, GSLOT], TBL_DT),
                ("norm", norm_d, [128, NBLK], mybir.dt.float32),
                ("norm2", norm2_d, [128, NBLK], mybir.dt.float32),
                ("w1", w1_d, [D, D], TBL_DT),
                ("w2", w2_d, [D, D], TBL_DT),
                ("w3", w3_d, [D, D_OUT], TBL_DT),
                ("b3", b3_d, [D_OUT, 1], mybir.dt.float32),
                ("ident", ident_d, [128, 128], TBL_DT),
                ("fidx", fidx_d, [128, 8], mybir.dt.int16),
            ):
                t = cpool.tile(shape, dt, tag=nm)
                nc.sync.dma_start(t[:], dr[:])
                consts[nm] = t

            idx_t = slabp.tile([128, plan.E_pad // 16], mybir.dt.int16,
                               tag="idx")
            nc.sync.dma_start(idx_t[:], idx_d[:])
            slot_t = slabp.tile([128, plan.E_pad // 128], mybir.dt.float32,
                                tag="slot")
            nc.sync.dma_start(slot_t[:], slot_d[:])

            pools = {"g": gp, "ind": indp, "agg": aggp, "aggsb": aggsbp,
                     "r": rp, "st": stp}
            slabs = (idx_t, slot_t)

            qn = _emit_layer(nc, plan, pools, consts, table0, S,
                             h1_bounce, consts["w1"], "norm2", slabs)

            nc.gpsimd.collective_compute(
                "AllGather", mybir.AluOpType.bypass,
                replica_groups=[list(range(NC))],
                ins=[h1_bounce.ap().opt()],
                outs=[table1.ap().opt()])

            _emit_layer(nc, plan, pools, consts, table1, S_PAD,
                        h2_loc, consts["w2"], "norm", slabs, qoff=qn)

            # final fetch + linear
            fcnt_reg = nc.gpsimd.alloc_register("fcnt_reg")
            nc.gpsimd.reg_load(fcnt_reg, fcnt_d[0:1, 0:1])
            fx = gp.tile([128, 1, D], TBL_DT, tag="g")
            nc.vector.memset(fx[:], 0.0)
            nc.gpsimd.dma_gather(fx[:], h2_loc[:], consts["fidx"][:],
                                 128, fcnt_reg, D)
            xt_ps = rp.tile([128, 128], TBL_DT, tag="r")
            nc.tensor.transpose(xt_ps[:], fx[:, 0, :], consts["ident"][:])
            xt = aggsbp.tile([128, 128], TBL_DT, tag="aggsb")
            nc.scalar.activation(xt[:], xt_ps[:],
                                 mybir.ActivationFunctionType.Copy)
            out_ps = rp.tile([D_OUT, 128], ACC_DT, tag="r")
            nc.tensor.matmul(out_ps[:], lhsT=consts["w3"][:], rhs=xt[:],
                             start=True, stop=True)
            out_sb = stp.tile([D_OUT, 128], mybir.dt.float32, tag="st")
            nc.vector.tensor_scalar_add(out_sb[:], out_ps[:],
                                        consts["b3"][:, 0:1])
            nc.sync.dma_start(y_d[:], out_sb[:])
    nc.compile()
    return nc


# ---------------------------------------------------------------- kernel


def kernel(features, src, dst, to_fetch, w1, b1, w2, b2, w3, b3):
    global last_result
    features = np.asarray(features)
    plan = Plan(np.asarray(src), np.asarray(dst))

    # fetch bookkeeping
    gidx = np.asarray(to_fetch).astype(np.int64) + \
        np.arange(NUM_GRAPHS, dtype=np.int64) * NODES_PER_GRAPH
    fown = gidx // S
    floc = gidx - fown * S
    fetch_rows = []   # per core: positions into the 100-row output
    fidx_arr = np.full((NC, 128), -1, np.int16)
    fcnt = np.zeros(NC, np.int64)
    for c in range(NC):
        rows = np.where(fown == c)[0]
        fetch_rows.append(rows)
        fidx_arr[c, : rows.size] = floc[rows].astype(np.int16)
        fcnt[c] = rows.size

    weights = (np.asarray(w1), np.asarray(b1), np.asarray(w2),
               np.asarray(b2), np.asarray(w3), np.asarray(b3))
    nc = build_bass(plan, weights)

    # ---- per-core inputs
    tbl0 = (features * plan.norm[:, None]).astype(TBL_NP)
    iota = np.tile(np.arange(GSLOT, dtype=TBL_NP)[None, :], (128, 1))
    ident = np.eye(128, dtype=TBL_NP)
    w1t = weights[0].astype(TBL_NP)          # [in, out] == lhs-free layout
    w2t = weights[2].astype(TBL_NP)
    w3t = weights[4].T.astype(TBL_NP)        # [128, 64]
    b3c = weights[5].reshape(D_OUT, 1).astype(np.float32)

    in_maps = []
    for c in range(NC):
        # dst-side norm per padded local block layout [p, blk]
        r = np.arange(S_PAD)
        vals = np.where(r < S, plan.norm[c * S + np.minimum(r, S - 1)], 1.0)
        nrm = vals.reshape(NBLK, 128).T.astype(np.float32)
        wrap16 = np.zeros((128, 8), np.int16)
        wrap16[:16] = fidx_arr[c].reshape(8, 16).T
        wrap16 = np.tile(wrap16[:16], (8, 1))
        in_maps.append({
            "table0": tbl0,
            "idx": plan.wrap_idx(c),
            "slot": plan.chunk_cols(plan.slot[c], np.float32),
            "iota": iota,
            "normsc": nrm,
            "normsc2": (nrm * nrm).astype(np.float32),
            "w1t": w1t, "w2t": w2t, "w3t": w3t, "b3c": b3c,
            "ident": ident,
            "fidx": wrap16,
            "fcnt": np.array([[fcnt[c]]], np.uint32),
        })

    res = run_bass_kernel_spmd(nc, in_maps, core_ids=list(range(NC)),
                               trace=bool(os.environ.get("BASS_TRACE")))
    last_result = res

    out = np.zeros((NUM_GRAPHS, D_OUT), np.float32)
    for c in range(NC):
        yc = res.results[c]["y"]  # [64, 128]
        rows = fetch_rows[c]
        out[rows] = yc[:, : rows.size].T
    return out


# revision 10
# speedup vs baseline: 1.0688x; 1.0688x over previous
"""GNN message-passing (2-layer GraphConv + fetch/linear) on 8 TRN2 NeuronCores.

Strategy (self-contained; shapes hardcoded for the target problem):
  - Nodes dst-sharded across 8 cores (25000/core). Each core owns the edges
    whose dst falls in its shard; the full feature table is replicated in
    DRAM so the per-edge gather h[src] is core-local.
  - The dst-side norm is folded into the tables on the host
    (table0 = features*norm; table1 = h1*norm by construction), so the
    per-chunk indicator is a plain is_equal (no per-edge norm multiply).
  - Edges are sorted per core by (group g of 8 superblocks, src-owner o,
    superblock sb of 256 dst slots). Only the (g, o) runs are padded (to the
    max across cores, 128-aligned) so all 8 cores run one SPMD program;
    128-edge chunks may span superblock boundaries inside a run. For each
    chunk x superblock incidence (the union across cores), one DVE is_equal
    builds a [128, 256] indicator from a group-local slot id, and one PE
    matmul accumulates aggT[f, slot] += X_chunk.T @ ind into that
    superblock's PSUM tile. Out-of-window slots simply never match, so a
    core lacking edges there contributes zero.
  - dma_gather (int16 indices, one 256B row per edge, up to 4096 rows per
    call) pulls src rows into SBUF. Index/slot slabs are SBUF-resident and
    shared by both layers.
  - Per 128-dst block: R = aggT_b.T @ W in PSUM, then one ScalarE
    activation relu(R * scale) writes the next-layer table row block
    (scale folds the dst-side norm; layer 1 stores h1*norm via scale=norm^2).
  - One AllGather replicates h1*norm to all cores between the layers.
    Final stage: dma_gather of the fetched rows, transpose, matmul with
    w3^T, bias.
"""

import os
import sys

sys.path.insert(0, "/opt/trn_rl_repo")

import numpy as np

import concourse.bacc as bacc
import concourse.bass as bass
import concourse.mybir as mybir
import concourse.tile as tile
from concourse.bass_utils import run_bass_kernel_spmd
from concourse.library_config import mlp as mlp_lib

# ---------------------------------------------------------------- config

N_NODES = 200000
N_EDGES = 3200000
NUM_GRAPHS = 100
NODES_PER_GRAPH = 2000
D = 128
D_OUT = 64
NC = 8                       # cores
S = N_NODES // NC            # 25000 dst nodes per core
SB = int(os.environ.get("SB", "256"))  # dst slots per superblock (PSUM bank)
NSB = (S + SB - 1) // SB
S_PAD = NSB * SB             # 25088 for SB in {256, 512}
G_SB = int(os.environ.get("G_SB", "4"))  # superblocks per gather group
NG = (NSB + G_SB - 1) // G_SB
GSLOT = G_SB * SB            # dst slots per group
NBLK = S_PAD // 128          # 196 dst blocks per core
NRUN = NG * NC               # (g, o) runs
MAX_CALL = int(os.environ.get("MAX_CALL", "1024"))  # rows per dma_gather call
N_QUEUES = 4

TBL_DT = mybir.dt.float16    # table / gather / indicator dtype
TBL_NP = np.float16
ACC_DT = mybir.dt.float32

last_result = None           # BassKernelResults of the most recent run


def _roundup(x, m):
    return (x + m - 1) // m * m


# ---------------------------------------------------------------- host plan


class Plan:
    """Host-side reorganization of the edge list into the SPMD structure."""

    def __init__(self, src, dst):
        src = np.asarray(src).astype(np.int64)
        dst = np.asarray(dst).astype(np.int64)
        deg = np.bincount(dst, minlength=N_NODES).astype(np.float64)
        self.norm = (1.0 / np.sqrt(np.clip(deg, 1.0, None))).astype(np.float32)

        owner = src // S
        core = dst // S
        dloc = dst - core * S
        sbg = dloc // SB                 # superblock 0..NSB-1
        g = sbg // G_SB                  # group 0..NG-1
        sbl = sbg - g * G_SB             # superblock within group
        slot_g = dloc - g * GSLOT        # group-local slot 0..GSLOT-1

        runid = g * NC + owner
        cellid = runid * G_SB + sbl
        ncell = NRUN * G_SB

        cellcnt = np.zeros((NC, ncell), np.int64)
        for c in range(NC):
            cellcnt[c] = np.bincount(cellid[core == c], minlength=ncell)
        runcnt = cellcnt.reshape(NC, NRUN, G_SB).sum(axis=2)
        target_run = _roundup(runcnt.max(axis=0), 128)          # [NRUN]
        run_base = np.zeros(NRUN, np.int64)
        run_base[1:] = np.cumsum(target_run)[:-1]
        self.E_pad = int(target_run.sum())
        self.target_run = target_run
        self.run_base = run_base

        # per-core cell start offsets inside the (padded, shared) run
        cell_off = np.zeros((NC, NRUN, G_SB), np.int64)
        cc = cellcnt.reshape(NC, NRUN, G_SB)
        cell_off[:, :, 1:] = np.cumsum(cc, axis=2)[:, :, :-1]
        cell_start = run_base[None, :, None] + cell_off        # [NC,NRUN,G_SB]
        cell_end = cell_start + cc

        # per-core padded edge arrays
        self.idx16 = np.zeros((NC, self.E_pad), np.int16)
        self.slot = np.full((NC, self.E_pad), -1.0, np.float32)
        starts_flat = cell_start.reshape(NC, ncell)
        for c in range(NC):
            m = core == c
            cid = cellid[m]
            srt = np.argsort(cid, kind="stable")
            cid_s = cid[srt]
            pref = np.zeros(ncell + 1, np.int64)
            pref[1:] = np.cumsum(cellcnt[c])
            ranks = np.arange(cid_s.size) - pref[cid_s]
            pos = starts_flat[c, cid_s] + ranks
            es = src[m][srt]
            self.idx16[c, pos] = (es - (es // S) * S).astype(np.int16)
            self.slot[c, pos] = slot_g[m][srt].astype(np.float32)

        # union chunk x superblock incidence (core-invariant op list)
        # ops[g] = ordered list of (t, sbl, is_first, is_last)
        nchunk = self.E_pad // 128
        inc = {}                          # (t, sbl) -> True
        for c in range(NC):
            st = cell_start[c]
            en = cell_end[c]
            for r in range(NRUN):
                for s_ in range(G_SB):
                    if en[r, s_] <= st[r, s_]:
                        continue
                    t0 = st[r, s_] // 128
                    t1 = (en[r, s_] + 127) // 128
                    for t in range(t0, t1):
                        inc[(t, s_)] = True

        chunk_run = np.zeros(nchunk, np.int64)
        for r in range(NRUN):
            c0 = run_base[r] // 128
            c1 = (run_base[r] + target_run[r]) // 128
            chunk_run[c0:c1] = r

        self.ops = [[] for _ in range(NG)]
        first_seen = {}
        last_idx = {}
        for gg in range(NG):
            seq = []
            for o in range(NC):
                r = gg * NC + o
                c0 = run_base[r] // 128
                c1 = (run_base[r] + target_run[r]) // 128
                for t in range(c0, c1):
                    for s_ in range(G_SB):
                        if (t, s_) in inc:
                            seq.append((t, s_))
            for i, (t, s_) in enumerate(seq):
                key = s_
                if key not in first_seen:
                    first_seen[key] = i
                last_idx[key] = i
            ops = []
            for i, (t, s_) in enumerate(seq):
                ops.append((t, s_, i == first_seen[s_], i == last_idx[s_]))
            self.ops[gg] = ops
            first_seen.clear()
            last_idx.clear()

        # gather calls: per run split into <=MAX_CALL pieces
        self.calls = [[] for _ in range(NG)]   # (o, start, n)
        for gg in range(NG):
            for o in range(NC):
                r = gg * NC + o
                start = int(run_base[r])
                left = int(target_run[r])
                while left > 0:
                    n = min(MAX_CALL, left)
                    self.calls[gg].append((o, start, n))
                    start += n
                    left -= n

        # chunk -> (call order index within group, k) for lhsT slicing
        self.chunk_call = {}
        for gg in range(NG):
            for ci, (o, start, n) in enumerate(self.calls[gg]):
                for k in range(n // 128):
                    self.chunk_call[start // 128 + k] = (gg, ci, k)

    def wrap_idx(self, c):
        a = self.idx16[c].reshape(-1, 16).T  # [16, E/16]
        return np.tile(a, (8, 1)).copy()

    def chunk_cols(self, arr_c, dtype):
        return arr_c.reshape(-1, 128).T.astype(dtype).copy()  # [128, E/128]


# ---------------------------------------------------------------- bass build


def _emit_layer(nc, plan, pools, consts, table, win_rows, bounce, w_tile,
                scale_name, slabs, qoff=0):
    gp, indp, aggp, aggsbp, rp, stp = (pools[k] for k in
                                       ("g", "ind", "agg", "aggsb", "r", "st"))
    iota_t = consts["iota"]
    scale_t = consts[scale_name]
    idx_t, slot_t = slabs

    qn = qoff
    for gg in range(NG):
        # gather calls for this group
        gtiles = []
        for (o, start, n) in plan.calls[gg]:
            gt = gp.tile([128, n // 128, D], TBL_DT, tag="g")
            nc.gpsimd.dma_gather(
                gt[:, : n // 128, :],
                table[o * win_rows: o * win_rows + win_rows, :],
                idx_t[:, start // 16: (start + n) // 16],
                n, n, D, queue_num=qn % N_QUEUES)
            qn += 1
            gtiles.append((start // 128, gt))

        agg_tiles = {}  # keyed by superblock; one PSUM bank each
        for (t, s_, is_first, is_last) in plan.ops[gg]:
            _, ci, k = plan.chunk_call[t]
            gt = gtiles[ci][1]
            ind = indp.tile([128, SB], TBL_DT, tag="ind")
            nc.vector.tensor_scalar(
                ind[:], iota_t[:, s_ * SB:(s_ + 1) * SB],
                slot_t[:, t:t + 1], None, mybir.AluOpType.is_equal)
            if s_ not in agg_tiles:
                agg_tiles[s_] = aggp.tile([128, SB], ACC_DT, tag="agg",
                                          name=f"agg_g{gg}_s{s_}")
            nc.tensor.matmul(agg_tiles[s_][:], lhsT=gt[:, k, :], rhs=ind[:],
                             start=is_first, stop=is_last)
            if is_last:
                # drain this superblock
                aggT = aggsbp.tile([128, SB], TBL_DT, tag="aggsb")
                nc.scalar.activation(aggT[:], agg_tiles[s_][:],
                                     mybir.ActivationFunctionType.Copy)
                stage = stp.tile([128, SB // 128, D], TBL_DT, tag="st")
                for b in range(SB // 128):
                    blk = (gg * GSLOT + s_ * SB) // 128 + b
                    r = rp.tile([128, D], ACC_DT, tag="r")
                    nc.tensor.matmul(r[:], lhsT=aggT[:, b * 128:(b + 1) * 128],
                                     rhs=w_tile[:], start=True, stop=True)
                    nc.scalar.activation(stage[:, b, :], r[:],
                                         mybir.ActivationFunctionType.Relu,
                                         scale=scale_t[:, blk:blk + 1])
                base = gg * GSLOT + s_ * SB
                nc.sync.dma_start(
                    bounce[base:base + SB, :].rearrange(
                        "(c p) f -> p c f", p=128),
                    stage[:])
    return qn


def build_bass(plan, weights):
    nc = bacc.Bacc("TRN2", target_bir_lowering=False,
                   num_swdge_queues=N_QUEUES)
    w1, b1, w2, b2, w3, b3 = weights
    assert abs(b1).max() == 0 and abs(b2).max() == 0, \
        "nonzero conv bias not supported by this build"

    E_pad = plan.E_pad
    table0 = nc.dram_tensor("table0", [N_NODES, D], TBL_DT,
                            kind="ExternalInput")
    idx_d = nc.dram_tensor("idx", [128, E_pad // 16], mybir.dt.int16,
                           kind="ExternalInput")
    slot_d = nc.dram_tensor("slot", [128, E_pad // 128], mybir.dt.float32,
                            kind="ExternalInput")
    iota_d = nc.dram_tensor("iota", [128, GSLOT], TBL_DT, kind="ExternalInput")
    norm_d = nc.dram_tensor("normsc", [128, NBLK], mybir.dt.float32,
                            kind="ExternalInput")
    norm2_d = nc.dram_tensor("normsc2", [128, NBLK], mybir.dt.float32,
                             kind="ExternalInput")
    w1_d = nc.dram_tensor("w1t", [D, D], TBL_DT, kind="ExternalInput")
    w2_d = nc.dram_tensor("w2t", [D, D], TBL_DT, kind="ExternalInput")
    w3_d = nc.dram_tensor("w3t", [D, D_OUT], TBL_DT, kind="ExternalInput")
    b3_d = nc.dram_tensor("b3c", [D_OUT, 1], mybir.dt.float32,
                          kind="ExternalInput")
    ident_d = nc.dram_tensor("ident", [128, 128], TBL_DT,
                             kind="ExternalInput")
    fidx_d = nc.dram_tensor("fidx", [128, 8], mybir.dt.int16,
                            kind="ExternalInput")
    fcnt_d = nc.dram_tensor("fcnt", [1, 1], mybir.dt.uint32,
                            kind="ExternalInput")
    y_d = nc.dram_tensor("y", [D_OUT, 128], mybir.dt.float32,
                         kind="ExternalOutput")

    h1_bounce = nc.dram_tensor("h1b", [S_PAD, D], TBL_DT)
    table1 = nc.dram_tensor("table1", [NC * S_PAD, D], TBL_DT,
                            addr_space="Shared")
    h2_loc = nc.dram_tensor("h2loc", [S_PAD, D], TBL_DT)

    with tile.TileContext(nc) as tc:
        nc.gpsimd.load_library(mlp_lib)
        with (
            tc.tile_pool(name="consts", bufs=1) as cpool,
            tc.tile_pool(name="slab", bufs=1) as slabp,
            tc.tile_pool(name="g", bufs=6) as gp,
            tc.tile_pool(name="ind", bufs=8) as indp,
            tc.tile_pool(name="aggsb", bufs=4) as aggsbp,
            tc.tile_pool(name="st", bufs=4) as stp,
            tc.tile_pool(name="agg", bufs=6, space="PSUM") as aggp,
            tc.tile_pool(name="r", bufs=2, space="PSUM") as rp,
        ):
            consts = {}
            for nm, dr, shape, dt in (
                ("iota", iota_d, [128, GSLOT], TBL_DT),
                ("norm", norm_d, [128, NBLK], mybir.dt.float32),
                ("norm2", norm2_d, [128, NBLK], mybir.dt.float32),
                ("w1", w1_d, [D, D], TBL_DT),
                ("w2", w2_d, [D, D], TBL_DT),
                ("w3", w3_d, [D, D_OUT], TBL_DT),
                ("b3", b3_d, [D_OUT, 1], mybir.dt.float32),
                ("ident", ident_d, [128, 128], TBL_DT),
                ("fidx", fidx_d, [128, 8], mybir.dt.int16),
            ):
                t = cpool.tile(shape, dt, tag=nm)
                nc.sync.dma_start(t[:], dr[:])
                consts[nm] = t

            idx_t = slabp.tile([128, plan.E_pad // 16], mybir.dt.int16,
                               tag="idx")
            nc.sync.dma_start(idx_t[:], idx_d[:])
            slot_t = slabp.tile([128, plan.E_pad // 128], mybir.dt.float32,
                                tag="slot")
            nc.sync.dma_start(slot_t[:], slot_d[:])

            pools = {"g": gp, "ind": indp, "agg": aggp, "aggsb": aggsbp,
                     "r": rp, "st": stp}
            slabs = (idx_t, slot_t)

            qn = _emit_layer(nc, plan, pools, consts, table0, S,
                             h1_bounce, consts["w1"], "norm2", slabs)

            nc.gpsimd.collective_compute(
                "AllGather", mybir.AluOpType.bypass,
                replica_groups=[list(range(NC))],
                ins=[h1_bounce.ap().opt()],
                outs=[table1.ap().opt()])

            _emit_layer(nc, plan, pools, consts, table1, S_PAD,
                        h2_loc, consts["w2"], "norm", slabs, qoff=qn)

            # final fetch + linear
            fcnt_reg = nc.gpsimd.alloc_register("fcnt_reg")
            nc.gpsimd.reg_load(fcnt_reg, fcnt_d[0:1, 0:1])
            fx = gp.tile([128, 1, D], TBL_DT, tag="g")
            nc.vector.memset(fx[:], 0.0)
            nc.gpsimd.dma_gather(fx[:], h2_loc[:], consts["fidx"][:],
                                 128, fcnt_reg, D)
            xt_ps = rp.tile([128, 128], TBL_DT, tag="r")
            nc.tensor.transpose(xt_ps[:], fx[:, 0, :], consts["ident"][:])
            xt = aggsbp.tile([128, 128], TBL_DT, tag="aggsb")
            nc.scalar.activation(xt[:], xt_ps[:],
                                 mybir.ActivationFunctionType.Copy)
            out_ps = rp.tile([D_OUT, 128], ACC_DT, tag="r")
            nc.tensor.matmul(out_ps[:], lhsT=consts["w3"][:], rhs=xt[:],
                             start=True, stop=True)
            out_sb = stp.tile([D_OUT, 128], mybir.dt.float32, tag="st")
            nc.vector.tensor_scalar_add(out_sb[:], out_ps[:],
                                        consts["b3"][:, 0:1])
            nc.sync.dma_start(y_d[:], out_sb[:])
    nc.compile()
    return nc


# ---------------------------------------------------------------- kernel


def kernel(features, src, dst, to_fetch, w1, b1, w2, b2, w3, b3):
    global last_result
    features = np.asarray(features)
    plan = Plan(np.asarray(src), np.asarray(dst))

    # fetch bookkeeping
    gidx = np.asarray(to_fetch).astype(np.int64) + \
        np.arange(NUM_GRAPHS, dtype=np.int64) * NODES_PER_GRAPH
    fown = gidx // S
    floc = gidx - fown * S
    fetch_rows = []   # per core: positions into the 100-row output
    fidx_arr = np.full((NC, 128), -1, np.int16)
    fcnt = np.zeros(NC, np.int64)
    for c in range(NC):
        rows = np.where(fown == c)[0]
        fetch_rows.append(rows)
        fidx_arr[c, : rows.size] = floc[rows].astype(np.int16)
        fcnt[c] = rows.size

    weights = (np.asarray(w1), np.asarray(b1), np.asarray(w2),
               np.asarray(b2), np.asarray(w3), np.asarray(b3))
    nc = build_bass(plan, weights)

    # ---- per-core inputs
    tbl0 = (features * plan.norm[:, None]).astype(TBL_NP)
    iota = np.tile(np.arange(GSLOT, dtype=TBL_NP)[None, :], (128, 1))
    ident = np.eye(128, dtype=TBL_NP)
    w1t = weights[0].astype(TBL_NP)          # [in, out] == lhs-free layout
    w2t = weights[2].astype(TBL_NP)
    w3t = weights[4].T.astype(TBL_NP)        # [128, 64]
    b3c = weights[5].reshape(D_OUT, 1).astype(np.float32)

    in_maps = []
    for c in range(NC):
        # dst-side norm per padded local block layout [p, blk]
        r = np.arange(S_PAD)
        vals = np.where(r < S, plan.norm[c * S + np.minimum(r, S - 1)], 1.0)
        nrm = vals.reshape(NBLK, 128).T.astype(np.float32)
        wrap16 = np.zeros((128, 8), np.int16)
        wrap16[:16] = fidx_arr[c].reshape(8, 16).T
        wrap16 = np.tile(wrap16[:16], (8, 1))
        in_maps.append({
            "table0": tbl0,
            "idx": plan.wrap_idx(c),
            "slot": plan.chunk_cols(plan.slot[c], np.float32),
            "iota": iota,
            "normsc": nrm,
            "normsc2": (nrm * nrm).astype(np.float32),
            "w1t": w1t, "w2t": w2t, "w3t": w3t, "b3c": b3c,
            "ident": ident,
            "fidx": wrap16,
            "fcnt": np.array([[fcnt[c]]], np.uint32),
        })

    res = run_bass_kernel_spmd(nc, in_maps, core_ids=list(range(NC)),
                               trace=bool(os.environ.get("BASS_TRACE")))
    last_result = res

    out = np.zeros((NUM_GRAPHS, D_OUT), np.float32)
    for c in range(NC):
        yc = res.results[c]["y"]  # [64, 128]
        rows = fetch_rows[c]
        out[rows] = yc[:, : rows.size].T
    return out


# revision 11
# speedup vs baseline: 1.3315x; 1.2458x over previous
"""GNN message-passing (2-layer GraphConv + fetch/linear) on 8 TRN2 NeuronCores.

Strategy (self-contained; shapes hardcoded for the target problem):
  - Nodes dst-sharded across 8 cores (25000/core). Each core owns the edges
    whose dst falls in its shard; the full feature table is replicated in
    DRAM so the per-edge gather h[src] is core-local.
  - The dst-side norm is folded into the tables on the host
    (table0 = features*norm; table1 = h1*norm by construction), so the
    per-chunk indicator is a plain is_equal (no per-edge norm multiply).
  - Edges are sorted per core by (group g of 8 superblocks, src-owner o,
    superblock sb of 256 dst slots). Only the (g, o) runs are padded (to the
    max across cores, 128-aligned) so all 8 cores run one SPMD program;
    128-edge chunks may span superblock boundaries inside a run. For each
    chunk x superblock incidence (the union across cores), one DVE is_equal
    builds a [128, 256] indicator from a group-local slot id, and one PE
    matmul accumulates aggT[f, slot] += X_chunk.T @ ind into that
    superblock's PSUM tile. Out-of-window slots simply never match, so a
    core lacking edges there contributes zero.
  - dma_gather (int16 indices, one 256B row per edge, up to 4096 rows per
    call) pulls src rows into SBUF. Index/slot slabs are SBUF-resident and
    shared by both layers.
  - Per 128-dst block: R = aggT_b.T @ W in PSUM, then one ScalarE
    activation relu(R * scale) writes the next-layer table row block
    (scale folds the dst-side norm; layer 1 stores h1*norm via scale=norm^2).
  - One AllGather replicates h1*norm to all cores between the layers.
    Final stage: dma_gather of the fetched rows, transpose, matmul with
    w3^T, bias.
"""

import os
import sys

sys.path.insert(0, "/opt/trn_rl_repo")

import numpy as np

import concourse.bacc as bacc
import concourse.bass as bass
import concourse.mybir as mybir
import concourse.tile as tile
from concourse.bass_utils import run_bass_kernel_spmd
from concourse.library_config import mlp as mlp_lib

# ---------------------------------------------------------------- config

N_NODES = 200000
N_EDGES = 3200000
NUM_GRAPHS = 100
NODES_PER_GRAPH = 2000
D = 128
D_OUT = 64
NC = 8                       # cores
S = N_NODES // NC            # 25000 dst nodes per core
SB = int(os.environ.get("SB", "256"))  # dst slots per superblock (PSUM bank)
NSB = (S + SB - 1) // SB
S_PAD = NSB * SB             # 25088 for SB in {256, 512}
G_SB = int(os.environ.get("G_SB", "6"))  # superblocks per gather group
NG = (NSB + G_SB - 1) // G_SB
GSLOT = G_SB * SB            # dst slots per group
NBLK = S_PAD // 128          # 196 dst blocks per core
NRUN = NG * NC               # (g, o) runs
MAX_CALL = int(os.environ.get("MAX_CALL", "1024"))  # rows per dma_gather call
N_QUEUES = 4

TBL_DT = mybir.dt.float16    # table / gather / indicator dtype
TBL_NP = np.float16
ACC_DT = mybir.dt.float32

last_result = None           # BassKernelResults of the most recent run


def _roundup(x, m):
    return (x + m - 1) // m * m


# ---------------------------------------------------------------- host plan


class Plan:
    """Host-side reorganization of the edge list into the SPMD structure."""

    def __init__(self, src, dst):
        src = np.asarray(src).astype(np.int64)
        dst = np.asarray(dst).astype(np.int64)
        deg = np.bincount(dst, minlength=N_NODES).astype(np.float64)
        self.norm = (1.0 / np.sqrt(np.clip(deg, 1.0, None))).astype(np.float32)

        owner = src // S
        core = dst // S
        dloc = dst - core * S
        sbg = dloc // SB                 # superblock 0..NSB-1
        g = sbg // G_SB                  # group 0..NG-1
        sbl = sbg - g * G_SB             # superblock within group
        slot_g = dloc - g * GSLOT        # group-local slot 0..GSLOT-1

        runid = g * NC + owner
        cellid = runid * G_SB + sbl
        ncell = NRUN * G_SB

        cellcnt = np.zeros((NC, ncell), np.int64)
        for c in range(NC):
            cellcnt[c] = np.bincount(cellid[core == c], minlength=ncell)
        runcnt = cellcnt.reshape(NC, NRUN, G_SB).sum(axis=2)
        target_run = _roundup(runcnt.max(axis=0), 128)          # [NRUN]
        run_base = np.zeros(NRUN, np.int64)
        run_base[1:] = np.cumsum(target_run)[:-1]
        self.E_pad = int(target_run.sum())
        self.target_run = target_run
        self.run_base = run_base

        # per-core cell start offsets inside the (padded, shared) run
        cell_off = np.zeros((NC, NRUN, G_SB), np.int64)
        cc = cellcnt.reshape(NC, NRUN, G_SB)
        cell_off[:, :, 1:] = np.cumsum(cc, axis=2)[:, :, :-1]
        cell_start = run_base[None, :, None] + cell_off        # [NC,NRUN,G_SB]
        cell_end = cell_start + cc

        # per-core padded edge arrays
        self.idx16 = np.zeros((NC, self.E_pad), np.int16)
        self.slot = np.full((NC, self.E_pad), -1.0, np.float32)
        starts_flat = cell_start.reshape(NC, ncell)
        for c in range(NC):
            m = core == c
            cid = cellid[m]
            srt = np.argsort(cid, kind="stable")
            cid_s = cid[srt]
            pref = np.zeros(ncell + 1, np.int64)
            pref[1:] = np.cumsum(cellcnt[c])
            ranks = np.arange(cid_s.size) - pref[cid_s]
            pos = starts_flat[c, cid_s] + ranks
            es = src[m][srt]
            self.idx16[c, pos] = (es - (es // S) * S).astype(np.int16)
            self.slot[c, pos] = slot_g[m][srt].astype(np.float32)

        # union chunk x superblock incidence (core-invariant op list)
        # ops[g] = ordered list of (t, sbl, is_first, is_last)
        nchunk = self.E_pad // 128
        inc = {}                          # (t, sbl) -> True
        for c in range(NC):
            st = cell_start[c]
            en = cell_end[c]
            for r in range(NRUN):
                for s_ in range(G_SB):
                    if en[r, s_] <= st[r, s_]:
                        continue
                    t0 = st[r, s_] // 128
                    t1 = (en[r, s_] + 127) // 128
                    for t in range(t0, t1):
                        inc[(t, s_)] = True

        chunk_run = np.zeros(nchunk, np.int64)
        for r in range(NRUN):
            c0 = run_base[r] // 128
            c1 = (run_base[r] + target_run[r]) // 128
            chunk_run[c0:c1] = r

        self.ops = [[] for _ in range(NG)]
        first_seen = {}
        last_idx = {}
        for gg in range(NG):
            seq = []
            for o in range(NC):
                r = gg * NC + o
                c0 = run_base[r] // 128
                c1 = (run_base[r] + target_run[r]) // 128
                for t in range(c0, c1):
                    for s_ in range(G_SB):
                        if (t, s_) in inc:
                            seq.append((t, s_))
            for i, (t, s_) in enumerate(seq):
                key = s_
                if key not in first_seen:
                    first_seen[key] = i
                last_idx[key] = i
            ops = []
            for i, (t, s_) in enumerate(seq):
                ops.append((t, s_, i == first_seen[s_], i == last_idx[s_]))
            self.ops[gg] = ops
            first_seen.clear()
            last_idx.clear()

        # gather calls: per run split into <=MAX_CALL pieces
        self.calls = [[] for _ in range(NG)]   # (o, start, n)
        for gg in range(NG):
            for o in range(NC):
                r = gg * NC + o
                start = int(run_base[r])
                left = int(target_run[r])
                while left > 0:
                    n = min(MAX_CALL, left)
                    self.calls[gg].append((o, start, n))
                    start += n
                    left -= n

        # chunk -> (call order index within group, k) for lhsT slicing
        self.chunk_call = {}
        for gg in range(NG):
            for ci, (o, start, n) in enumerate(self.calls[gg]):
                for k in range(n // 128):
                    self.chunk_call[start // 128 + k] = (gg, ci, k)

    def wrap_idx(self, c):
        a = self.idx16[c].reshape(-1, 16).T  # [16, E/16]
        return np.tile(a, (8, 1)).copy()

    def chunk_cols(self, arr_c, dtype):
        return arr_c.reshape(-1, 128).T.astype(dtype).copy()  # [128, E/128]


# ---------------------------------------------------------------- bass build


def _emit_layer(nc, plan, pools, consts, table, win_rows, bounce, w_tile,
                scale_name, slabs, qoff=0):
    gp, indp, aggp, aggsbp, rp, stp = (pools[k] for k in
                                       ("g", "ind", "agg", "aggsb", "r", "st"))
    iota_t = consts["iota"]
    scale_t = consts[scale_name]
    idx_t, slot_t = slabs

    qn = qoff
    for gg in range(NG):
        # gather calls for this group
        gtiles = []
        for (o, start, n) in plan.calls[gg]:
            gt = gp.tile([128, n // 128, D], TBL_DT, tag="g")
            nc.gpsimd.dma_gather(
                gt[:, : n // 128, :],
                table[o * win_rows: o * win_rows + win_rows, :],
                idx_t[:, start // 16: (start + n) // 16],
                n, n, D, queue_num=qn % N_QUEUES)
            qn += 1
            gtiles.append((start // 128, gt))

        agg_tiles = {}  # keyed by superblock; one PSUM bank each
        for (t, s_, is_first, is_last) in plan.ops[gg]:
            _, ci, k = plan.chunk_call[t]
            gt = gtiles[ci][1]
            ind = indp.tile([128, SB], TBL_DT, tag="ind")
            nc.vector.tensor_scalar(
                ind[:], iota_t[:, s_ * SB:(s_ + 1) * SB],
                slot_t[:, t:t + 1], None, mybir.AluOpType.is_equal)
            if s_ not in agg_tiles:
                agg_tiles[s_] = aggp.tile([128, SB], ACC_DT, tag="agg",
                                          name=f"agg_g{gg}_s{s_}")
            nc.tensor.matmul(agg_tiles[s_][:], lhsT=gt[:, k, :], rhs=ind[:],
                             start=is_first, stop=is_last)
            if is_last:
                # drain this superblock
                aggT = aggsbp.tile([128, SB], TBL_DT, tag="aggsb")
                nc.scalar.activation(aggT[:], agg_tiles[s_][:],
                                     mybir.ActivationFunctionType.Copy)
                stage = stp.tile([128, SB // 128, D], TBL_DT, tag="st")
                for b in range(SB // 128):
                    blk = (gg * GSLOT + s_ * SB) // 128 + b
                    r = rp.tile([128, D], ACC_DT, tag="r")
                    nc.tensor.matmul(r[:], lhsT=aggT[:, b * 128:(b + 1) * 128],
                                     rhs=w_tile[:], start=True, stop=True)
                    nc.scalar.activation(stage[:, b, :], r[:],
                                         mybir.ActivationFunctionType.Relu,
                                         scale=scale_t[:, blk:blk + 1])
                base = gg * GSLOT + s_ * SB
                nc.sync.dma_start(
                    bounce[base:base + SB, :].rearrange(
                        "(c p) f -> p c f", p=128),
                    stage[:])
    return qn


def build_bass(plan, weights):
    nc = bacc.Bacc("TRN2", target_bir_lowering=False,
                   num_swdge_queues=N_QUEUES)
    w1, b1, w2, b2, w3, b3 = weights
    assert abs(b1).max() == 0 and abs(b2).max() == 0, \
        "nonzero conv bias not supported by this build"

    E_pad = plan.E_pad
    table0 = nc.dram_tensor("table0", [N_NODES, D], TBL_DT,
                            kind="ExternalInput")
    idx_d = nc.dram_tensor("idx", [128, E_pad // 16], mybir.dt.int16,
                           kind="ExternalInput")
    slot_d = nc.dram_tensor("slot", [128, E_pad // 128], mybir.dt.float32,
                            kind="ExternalInput")
    iota_d = nc.dram_tensor("iota", [128, GSLOT], TBL_DT, kind="ExternalInput")
    norm_d = nc.dram_tensor("normsc", [128, NBLK], mybir.dt.float32,
                            kind="ExternalInput")
    norm2_d = nc.dram_tensor("normsc2", [128, NBLK], mybir.dt.float32,
                             kind="ExternalInput")
    w1_d = nc.dram_tensor("w1t", [D, D], TBL_DT, kind="ExternalInput")
    w2_d = nc.dram_tensor("w2t", [D, D], TBL_DT, kind="ExternalInput")
    w3_d = nc.dram_tensor("w3t", [D, D_OUT], TBL_DT, kind="ExternalInput")
    b3_d = nc.dram_tensor("b3c", [D_OUT, 1], mybir.dt.float32,
                          kind="ExternalInput")
    ident_d = nc.dram_tensor("ident", [128, 128], TBL_DT,
                             kind="ExternalInput")
    fidx_d = nc.dram_tensor("fidx", [128, 8], mybir.dt.int16,
                            kind="ExternalInput")
    fcnt_d = nc.dram_tensor("fcnt", [1, 1], mybir.dt.uint32,
                            kind="ExternalInput")
    y_d = nc.dram_tensor("y", [D_OUT, 128], mybir.dt.float32,
                         kind="ExternalOutput")

    h1_bounce = nc.dram_tensor("h1b", [S_PAD, D], TBL_DT)
    table1 = nc.dram_tensor("table1", [NC * S_PAD, D], TBL_DT,
                            addr_space="Shared")
    h2_loc = nc.dram_tensor("h2loc", [S_PAD, D], TBL_DT)

    with tile.TileContext(nc) as tc:
        nc.gpsimd.load_library(mlp_lib)
        with (
            tc.tile_pool(name="consts", bufs=1) as cpool,
            tc.tile_pool(name="slab", bufs=1) as slabp,
            tc.tile_pool(name="g", bufs=6) as gp,
            tc.tile_pool(name="ind", bufs=8) as indp,
            tc.tile_pool(name="aggsb", bufs=4) as aggsbp,
            tc.tile_pool(name="st", bufs=4) as stp,
            tc.tile_pool(name="agg", bufs=6, space="PSUM") as aggp,
            tc.tile_pool(name="r", bufs=2, space="PSUM") as rp,
        ):
            consts = {}
            for nm, dr, shape, dt in (
                ("iota", iota_d, [128, GSLOT], TBL_DT),
                ("norm", norm_d, [128, NBLK], mybir.dt.float32),
                ("norm2", norm2_d, [128, NBLK], mybir.dt.float32),
                ("w1", w1_d, [D, D], TBL_DT),
                ("w2", w2_d, [D, D], TBL_DT),
                ("w3", w3_d, [D, D_OUT], TBL_DT),
                ("b3", b3_d, [D_OUT, 1], mybir.dt.float32),
                ("ident", ident_d, [128, 128], TBL_DT),
                ("fidx", fidx_d, [128, 8], mybir.dt.int16),
            ):
                t = cpool.tile(shape, dt, tag=nm)
                nc.sync.dma_start(t[:], dr[:])
                consts[nm] = t

            idx_t = slabp.tile([128, plan.E_pad // 16], mybir.dt.int16,
                               tag="idx")
            nc.sync.dma_start(idx_t[:], idx_d[:])
            slot_t = slabp.tile([128, plan.E_pad // 128], mybir.dt.float32,
                                tag="slot")
            nc.sync.dma_start(slot_t[:], slot_d[:])

            pools = {"g": gp, "ind": indp, "agg": aggp, "aggsb": aggsbp,
                     "r": rp, "st": stp}
            slabs = (idx_t, slot_t)

            qn = _emit_layer(nc, plan, pools, consts, table0, S,
                             h1_bounce, consts["w1"], "norm2", slabs)

            nc.gpsimd.collective_compute(
                "AllGather", mybir.AluOpType.bypass,
                replica_groups=[list(range(NC))],
                ins=[h1_bounce.ap().opt()],
                outs=[table1.ap().opt()])

            _emit_layer(nc, plan, pools, consts, table1, S_PAD,
                        h2_loc, consts["w2"], "norm", slabs, qoff=qn)

            # final fetch + linear
            fcnt_reg = nc.gpsimd.alloc_register("fcnt_reg")
            nc.gpsimd.reg_load(fcnt_reg, fcnt_d[0:1, 0:1])
            fx = gp.tile([128, 1, D], TBL_DT, tag="g")
            nc.vector.memset(fx[:], 0.0)
            nc.gpsimd.dma_gather(fx[:], h2_loc[:], consts["fidx"][:],
                                 128, fcnt_reg, D)
            xt_ps = rp.tile([128, 128], TBL_DT, tag="r")
            nc.tensor.transpose(xt_ps[:], fx[:, 0, :], consts["ident"][:])
            xt = aggsbp.tile([128, 128], TBL_DT, tag="aggsb")
            nc.scalar.activation(xt[:], xt_ps[:],
                                 mybir.ActivationFunctionType.Copy)
            out_ps = rp.tile([D_OUT, 128], ACC_DT, tag="r")
            nc.tensor.matmul(out_ps[:], lhsT=consts["w3"][:], rhs=xt[:],
                             start=True, stop=True)
            out_sb = stp.tile([D_OUT, 128], mybir.dt.float32, tag="st")
            nc.vector.tensor_scalar_add(out_sb[:], out_ps[:],
                                        consts["b3"][:, 0:1])
            nc.sync.dma_start(y_d[:], out_sb[:])
    nc.compile()
    return nc


# ---------------------------------------------------------------- kernel


def kernel(features, src, dst, to_fetch, w1, b1, w2, b2, w3, b3):
    global last_result
    features = np.asarray(features)
    plan = Plan(np.asarray(src), np.asarray(dst))

    # fetch bookkeeping
    gidx = np.asarray(to_fetch).astype(np.int64) + \
        np.arange(NUM_GRAPHS, dtype=np.int64) * NODES_PER_GRAPH
    fown = gidx // S
    floc = gidx - fown * S
    fetch_rows = []   # per core: positions into the 100-row output
    fidx_arr = np.full((NC, 128), -1, np.int16)
    fcnt = np.zeros(NC, np.int64)
    for c in range(NC):
        rows = np.where(fown == c)[0]
        fetch_rows.append(rows)
        fidx_arr[c, : rows.size] = floc[rows].astype(np.int16)
        fcnt[c] = rows.size

    weights = (np.asarray(w1), np.asarray(b1), np.asarray(w2),
               np.asarray(b2), np.asarray(w3), np.asarray(b3))
    nc = build_bass(plan, weights)

    # ---- per-core inputs
    tbl0 = (features * plan.norm[:, None]).astype(TBL_NP)
    iota = np.tile(np.arange(GSLOT, dtype=TBL_NP)[None, :], (128, 1))
    ident = np.eye(128, dtype=TBL_NP)
    w1t = weights[0].astype(TBL_NP)          # [in, out] == lhs-free layout
    w2t = weights[2].astype(TBL_NP)
    w3t = weights[4].T.astype(TBL_NP)        # [128, 64]
    b3c = weights[5].reshape(D_OUT, 1).astype(np.float32)

    in_maps = []
    for c in range(NC):
        # dst-side norm per padded local block layout [p, blk]
        r = np.arange(S_PAD)
        vals = np.where(r < S, plan.norm[c * S + np.minimum(r, S - 1)], 1.0)
        nrm = vals.reshape(NBLK, 128).T.astype(np.float32)
        wrap16 = np.zeros((128, 8), np.int16)
        wrap16[:16] = fidx_arr[c].reshape(8, 16).T
        wrap16 = np.tile(wrap16[:16], (8, 1))
        in_maps.append({
            "table0": tbl0,
            "idx": plan.wrap_idx(c),
            "slot": plan.chunk_cols(plan.slot[c], np.float32),
            "iota": iota,
            "normsc": nrm,
            "normsc2": (nrm * nrm).astype(np.float32),
            "w1t": w1t, "w2t": w2t, "w3t": w3t, "b3c": b3c,
            "ident": ident,
            "fidx": wrap16,
            "fcnt": np.array([[fcnt[c]]], np.uint32),
        })

    res = run_bass_kernel_spmd(nc, in_maps, core_ids=list(range(NC)),
                               trace=bool(os.environ.get("BASS_TRACE")))
    last_result = res

    out = np.zeros((NUM_GRAPHS, D_OUT), np.float32)
    for c in range(NC):
        yc = res.results[c]["y"]  # [64, 128]
        rows = fetch_rows[c]
        out[rows] = yc[:, : rows.size].T
    return out
